# revision 38
# baseline (speedup 1.0000x reference)
"""Causal self-attention (B=4, T=2048, C=768, H=12) on 8 TRN2 NeuronCores.

Sharding: (batch x head-half). Core c handles batch b = c//2 and heads
hg*6..hg*6+5 where hg = c%2. Each core computes the qkv projection for its
1152 W_attn columns, causal attention for its 6 heads, and a partial
c_proj using its 384 rows of W_proj. Host sums the pair partials + b_eff.

Software-pipelined single-stream structure (~200us, vs 248us baseline):
- Query chunks processed in order [0, 3, 2, 1]: chunk 0 first (smallest
  DMA prerequisite set -> earliest start), the exp-heaviest chunk second,
  a light chunk last (ScalarE exp deficit never leaves the PE idle at the
  end).
- Q/K/V/c_proj are decoupled into small work items emitted just-in-time
  before the attention pti that needs them, or earlier as "fillers" paced
  by a debt ledger into the ACT-bound attention inner loop.
- Score matmuls of a head pair (h2=0 on PE rows 0-63, h2=1 on rows
  64-127) write one shared PSUM tile (h2=1 at column 512) so both release
  on the same exp and issue back-to-back -> they overlap in the PE array.
- PV matmuls are deferred TWO ptis so their pt operands (exp + causal
  mask) are always long done -> no pipeline-restart penalty.
- Pair normalization is deferred one pair and flushed after the next
  pair's jit projections: its vector-queue ops never delay the qkT casts
  (cross-engine deps are queue-prefix-ordered counting semaphores).
- Causal masking of diagonal blocks = VectorE multiply with a
  precomputed 0/1 triangle (gpsimd's queue would otherwise gate pt reuse
  behind slow partition_broadcasts).
- wqk is stored pair-major host-side and the input DMAs are staged
  (pair-0 weights + first xt chunk first) so the first matmul starts
  ~8us in instead of ~18us.
- Bias algebra: K-projection bias dropped (softmax-invariant), V bias
  folded host-side into b_eff = b_proj + b_v @ W_proj, Q bias enters as
  exp(bqK/8) folded into the PV stationary operand (col 64 of the
  stationary = scaled ones -> row 64 of the PV accumulator is the softmax
  denominator).
- c_proj is split into two 384-wide halves (1 PSUM bank each), deferred
  one chunk as filler work; the last chunk's c_proj is software-pipelined
  so pair-0/1 partials run under the final normalization wait.
"""

import sys

import numpy as np

try:
    import concourse  # noqa: F401
except ImportError:
    sys.path.insert(0, "/opt/trn_rl_repo")

B, T, C, H, D = 4, 2048, 768, 12, 64
NH = H // 2          # 6 heads per core
CH = NH * D          # 384 channels per core
NCB = C // 128       # 6 contraction blocks
NTB = T // 128       # 16 t-blocks
NQC = T // 512       # 4 query chunks
NPAIR = NH // 2      # 3 head pairs
VW2 = D + 2          # 66: [V(64), eb, pad] per head (col 64 = eb = exp(bqK/8))
VROW = NH * VW2      # 396
CORDER = [0, 3, 2, 1]

_CACHE = {}


def _build_nc():
    from concourse import bacc, mybir, tile

    f32 = mybir.dt.float32
    bf16 = mybir.dt.bfloat16
    AF = mybir.ActivationFunctionType
    ALU = mybir.AluOpType

    nc = bacc.Bacc("TRN2", target_bir_lowering=False, debug=False, num_devices=8)

    xt_d = nc.dram_tensor("xt", [C, T], bf16, kind="ExternalInput")
    wqk_d = nc.dram_tensor("wqk", [C, 2 * CH], bf16, kind="ExternalInput")
    wv_d = nc.dram_tensor("wv", [C, CH + NH], bf16, kind="ExternalInput")
    wp_d = nc.dram_tensor("wp", [128, NPAIR * C], bf16, kind="ExternalInput")
    out_d = nc.dram_tensor("out", [T, C], f32, kind="ExternalOutput")

    with tile.TileContext(nc) as tc:
        with (
            tc.tile_pool(name="const", bufs=1) as cp,
            tc.tile_pool(name="wk", bufs=3) as wk,
            tc.tile_pool(name="pt", bufs=4) as ptp,
            tc.tile_pool(name="ot", bufs=2) as otp,
            tc.tile_pool(name="outs", bufs=2) as osp,
            tc.tile_pool(name="ps", bufs=2, space="PSUM") as psS,
            tc.tile_pool(name="pj", bufs=2, space="PSUM") as psP,
            tc.tile_pool(name="pv", bufs=2, space="PSUM") as psV,
        ):
            # ---- resident inputs (full-width rows: max DMA run length) ----
            xt_r = xt_d.rearrange("(n p) m -> n p m", p=128)
            wqk_r = wqk_d.rearrange("(n p) m -> n p m", p=128)
            wv_r = wv_d.rearrange("(n p) m -> n p m", p=128)
            # wqk is PAIR-MAJOR host-side: [Q0|K0|Q1|K1|Q2|K2] x128 cols.
            # Stream order: pair-0 weights + first query chunk of xt first so
            # the projection pipeline starts early and is paced per ci block.
            xt_t, wqk_t, wv_t = [], [], []
            for ci in range(NCB):
                t_ = cp.tile([128, 2 * CH], bf16, tag=f"wqk{ci}", name=f"wqk{ci}")
                nc.sync.dma_start(out=t_[:, 0:256], in_=wqk_r[ci][:, 0:256])
                wqk_t.append(t_)
                t_ = cp.tile([128, T], bf16, tag=f"xt{ci}", name=f"xt{ci}")
                nc.sync.dma_start(out=t_[:, 0:512], in_=xt_r[ci][:, 0:512])
                xt_t.append(t_)
            for ci in range(NCB):
                t_ = cp.tile([128, CH + NH], bf16, tag=f"wv{ci}", name=f"wv{ci}")
                nc.sync.dma_start(out=t_, in_=wv_r[ci])
                wv_t.append(t_)
            for ci in range(NCB):
                nc.sync.dma_start(out=wqk_t[ci][:, 256:512],
                                  in_=wqk_r[ci][:, 256:512])
            for ci in range(NCB):
                nc.sync.dma_start(out=wqk_t[ci][:, 512:768],
                                  in_=wqk_r[ci][:, 512:768])
            for ci in range(NCB):
                nc.sync.dma_start(out=xt_t[ci][:, 512:T],
                                  in_=xt_r[ci][:, 512:T])
            wp_sb = cp.tile([128, NPAIR, C], bf16, tag="wp", name="wp")
            nc.sync.dma_start(out=wp_sb, in_=wp_d.rearrange("p (n m) -> p n m", n=NPAIR))

            def wqk_blk(blk, ci):
                return wqk_t[ci][:, blk * 128:(blk + 1) * 128]

            def xt_cols(ci, c0, c1):
                return xt_t[ci][:, c0:c1]

            # causal 0/1 mask for diagonal 128x128 blocks, built once so the
            # per-block masking runs on VectorE (gpsimd's prefix-ordered queue
            # otherwise gates pt reuse behind slow partition_broadcasts)
            tri = cp.tile([128, 128], bf16, tag="tri", name="tri")
            nc.gpsimd.memset(tri, 1.0)
            nc.gpsimd.affine_select(
                out=tri, in_=tri, compare_op=ALU.is_ge, fill=0.0, base=0,
                pattern=[[1, 128]], channel_multiplier=-1,
            )

            qkT = cp.tile([128, 6, T], bf16, tag="qkT", name="qkT")  # 0-2: Q, 3-5: K
            v1 = cp.tile([128, NTB, VROW], bf16, tag="v1", name="v1")
            v1_4d = v1.rearrange("p n (h e) -> p n h e", e=VW2)

            # ---------- work items ----------
            done_qk = {}       # (co, tc) -> True   co 0-2 Q-pair, 3-5 K-pair
            done_v = {}        # tb -> True

            def emit_proj(co, tcn):
                """Q or K projection for pair-column co, token chunk tcn."""
                if done_qk.get((co, tcn)):
                    return 0
                done_qk[(co, tcn)] = True
                blk = 2 * co if co < NPAIR else 2 * (co - NPAIR) + 1
                ps = psP.tile([128, 512], f32, tag="pj", name="pspj")
                for ci in range(NCB):
                    nc.tensor.matmul(
                        ps,
                        lhsT=wqk_blk(blk, ci),
                        rhs=xt_cols(ci, tcn * 512, (tcn + 1) * 512),
                        start=(ci == 0),
                        stop=(ci == NCB - 1),
                    )
                nc.vector.tensor_copy(qkT[:, co, tcn * 512:(tcn + 1) * 512], ps)
                return 1450

            def emit_v(tb):
                """V (+ bqK) projection for key t-block tb."""
                if done_v.get(tb):
                    return 0
                done_v[tb] = True
                psv = psP.tile([128, 512], f32, tag="pj", name="pspj")
                for ci in range(NCB):
                    nc.tensor.matmul(
                        psv[:, 0:CH + NH],
                        lhsT=xt_cols(ci, tb * 128, (tb + 1) * 128),
                        rhs=wv_t[ci],
                        start=(ci == 0),
                        stop=(ci == NCB - 1),
                    )
                eb = wk.tile([128, NH], f32, tag="eb", name="eb")
                nc.scalar.activation(eb, psv[:, CH:CH + NH], AF.Exp, scale=0.125)
                eb3 = eb.rearrange("p (h o) -> p h o", o=1)
                nc.vector.tensor_mul(
                    v1_4d[:, tb, :, 0:D],
                    psv[:, 0:CH].rearrange("p (h e) -> p h e", e=D),
                    eb3.to_broadcast([128, NH, D]),
                )
                nc.vector.tensor_copy(v1_4d[:, tb, :, D:D + 1], eb3)
                return 1150

            def emit_cproj_half(c, tb4, half, ot_tiles):
                """c_proj for t-block c*4+tb4, output columns half*384:+384."""
                tb = c * 4 + tb4
                pp = psP.tile([128, 512], f32, tag="pj", name="pspj")
                for p in range(NPAIR):
                    nc.tensor.matmul(
                        pp[:, 0:CH],
                        lhsT=ot_tiles[p][:, tb4 * 128:(tb4 + 1) * 128],
                        rhs=wp_sb[:, p, half * CH:(half + 1) * CH],
                        start=(p == 0),
                        stop=(p == NPAIR - 1),
                    )
                ost = osp.tile([128, CH], f32, tag=f"ost{half}", name="ost")
                nc.vector.tensor_copy(ost, pp[:, 0:CH])
                nc.sync.dma_start(
                    out=out_d[tb * 128:(tb + 1) * 128, half * CH:(half + 1) * CH],
                    in_=ost,
                )
                return 680

            cfill = []         # c_proj closures (must drain one chunk ahead)
            fillers = []       # proj/V closures returning pe-ns
            debt = [0.0]
            ot_map = {}        # chunk -> [otpair per pair]
            norm_pending = [None]

            def flush_norm():
                if norm_pending[0] is None:
                    return
                ps_pv, ots, p = norm_pending[0]
                norm_pending[0] = None
                otpair = otp.tile([128, 512], bf16, tag=f"ot{p}",
                                  name=f"ot{p}")
                for h2 in range(2):
                    # sums live at partition 64; custom-DVE ops misread
                    # base-64 APs, so standard-copy to partition 0 first
                    sums_sb = wk.tile([1, 512], f32, tag="sums", name="sums")
                    nc.vector.tensor_copy(sums_sb, ps_pv[h2][D:D + 1, :])
                    rb1 = wk.tile([1, 512], f32, tag="rb1", name="rb1")
                    nc.vector.reciprocal_approx_fast(rb1, sums_sb)
                    rbb = wk.tile([64, 512], f32, tag="rbb", name="rbb")
                    nc.gpsimd.partition_broadcast(rbb, rb1)
                    nc.vector.tensor_mul(
                        otpair[h2 * 64:(h2 + 1) * 64, :],
                        ps_pv[h2][0:D, :],
                        rbb,
                    )
                ots[p] = otpair

            def pump(ns):
                debt[0] += ns
                while debt[0] > 0 and (cfill or fillers):
                    q = cfill if cfill else fillers
                    debt[0] -= q.pop(0)()

            def emit_pv(nc_, pts, ps_pv, p, pti, j, nkb):
                for half, kb in ((0, 2 * pti), (1, 2 * pti + 1)):
                    pt, wd = pts[half]
                    qlo = 512 - wd
                    for h2 in range(2):
                        nc_.tensor.matmul(
                            ps_pv[h2][0:1 + D, qlo:512],
                            lhsT=v1_4d[:, kb, 2 * p + h2, 0:1 + D],
                            rhs=pt[:, h2 * 512:h2 * 512 + wd],
                            start=(kb == 0),
                            stop=(kb == nkb - 1),
                        )

            # ---------- main stream ----------
            pending = None     # (chunk, ot_tiles) whose c_proj is deferred
            for ic, j in enumerate(CORDER):
                nkb = 4 * (j + 1)
                # c_proj fillers of the chunk-before-last MUST be fully
                # emitted before this chunk's normalization recycles the ot
                # buffers (otp bufs=2), else PE-queue/WAR cycle -> deadlock
                for f in cfill:
                    f()
                cfill.clear()
                # queue fillers: deferred c_proj, then next chunk's prereqs
                if pending is not None:
                    pc, pots = pending
                    for tb4 in range(4):
                        for half in range(2):
                            cfill.append(
                                lambda c=pc, t=tb4, hf=half, o=pots:
                                emit_cproj_half(c, t, hf, o)
                            )
                    pending = None
                if ic + 1 < len(CORDER):
                    cn = CORDER[ic + 1]
                    for p in range(NPAIR):
                        for tcn in range(cn + 1):
                            if not done_qk.get((3 + p, tcn)):
                                fillers.append(
                                    lambda co=3 + p, t=tcn: emit_proj(co, t))
                        if not done_qk.get((p, cn)):
                            fillers.append(lambda co=p, t=cn: emit_proj(co, t))
                    for tb in range(4 * (cn + 1)):
                        if not done_v.get(tb):
                            fillers.append(lambda t=tb: emit_v(t))

                ot_map[j] = [None] * NPAIR
                for p in range(NPAIR):
                    # jit prereqs for this pair
                    debt[0] -= emit_proj(p, j)
                    for tcn in range(j + 1):
                        debt[0] -= emit_proj(3 + p, tcn)
                    flush_norm()
                    ps_pv = [
                        psV.tile([128, 512], f32, tag="pv", name=f"pspv{h2}")
                        for h2 in range(2)
                    ]
                    prevq = []   # pt tiles awaiting PV, deferred TWO ptis
                    pw = 1024.0
                    for pti in range(nkb // 2):
                        kb0, kb1 = 2 * pti, 2 * pti + 1
                        d0 = kb0 * 128 - j * 512
                        d1 = d0 + 128
                        qlo0, qlo1 = max(d0, 0), max(d1, 0)
                        w0, w1 = 512 - qlo0, 512 - qlo1
                        # one psum tile per key block holding BOTH h2 halves
                        # (h2=1 at column 512) so the paired score matmuls
                        # release together and issue back-to-back
                        pss = [
                            psS.tile([128, 1024], f32, tag="s", name=f"pss{kk}")
                            for kk in range(2)
                        ]
                        for kk, (kb, wd, ql) in enumerate(
                                ((kb0, w0, qlo0), (kb1, w1, qlo1))):
                            for h2 in range(2):
                                hp = h2 * 64
                                nc.tensor.matmul(
                                    pss[kk][:, h2 * 512:h2 * 512 + wd],
                                    lhsT=qkT[hp:hp + 64, 3 + p,
                                             kb * 128:(kb + 1) * 128],
                                    rhs=qkT[hp:hp + 64, p,
                                            j * 512 + ql:(j + 1) * 512],
                                    start=True,
                                    stop=True,
                                )
                        cur = []
                        for kk, (kb, wd, dd) in enumerate(
                                ((kb0, w0, d0), (kb1, w1, d1))):
                            pt = ptp.tile([128, 1024], bf16, tag=f"pt{kk}",
                                          name=f"pt{kk}")
                            if wd == 512:
                                nc.scalar.activation(
                                    pt, pss[kk], AF.Exp, scale=0.125)
                            else:
                                for h2 in range(2):
                                    nc.scalar.activation(
                                        pt[:, h2 * 512:h2 * 512 + wd],
                                        pss[kk][:, h2 * 512:h2 * 512 + wd],
                                        AF.Exp, scale=0.125,
                                    )
                            if dd >= 0:
                                for h2 in range(2):
                                    nc.vector.tensor_mul(
                                        pt[:, h2 * 512:h2 * 512 + 128],
                                        pt[:, h2 * 512:h2 * 512 + 128],
                                        tri,
                                    )
                            cur.append((pt, wd))
                        # V for these key blocks (consumed by a LATER pti's
                        # PV): emitted after the scores so the exp stream is
                        # never blocked behind V-proj (which gates on wv DMA)
                        debt[0] -= emit_v(kb0)
                        debt[0] -= emit_v(kb1)
                        # deficit: ACT exp time minus attention PE time this pti
                        sw = w0 + w1
                        prevq.append((cur, pti))
                        # drain to depth 1 at the pair's last pti: halves the
                        # pair-end PV backlog so the normalization chain (which
                        # gates the next pair's PV start via the psV buffer
                        # WAR) begins earlier
                        limit = 1 if pti == nkb // 2 - 1 else 2
                        popped = False
                        while len(prevq) > limit:
                            pts, pv_pti = prevq.pop(0)
                            emit_pv(nc, pts, ps_pv, p, pv_pti, j, nkb)
                            popped = True
                        if popped:
                            pump(1.25 * sw + 358 - 0.833 * pw)
                        else:
                            pump(1.25 * sw + 358)
                        pw = sw
                    for pts, pv_pti in prevq:
                        emit_pv(nc, pts, ps_pv, p, pv_pti, j, nkb)
                    # normalization is deferred one pair (flushed after the
                    # NEXT pair's jit projections) so its vector-queue ops
                    # never block the qkT casts the next scores need
                    norm_pending[0] = (ps_pv, ot_map[j], p)
                pending = (j, ot_map[j])

            # drain leftovers, then the last chunk's c_proj:
            # software-pipelined so each item's pair-0/1 matmuls run while
            # the previous item waits on the final pair-2 normalization
            flush_norm()
            for f in cfill:
                f()
            for f in fillers:
                f()
            pc, pots = pending

            def cfinish(item):
                pp_, tb4_, half_ = item
                nc.tensor.matmul(
                    pp_[:, 0:CH],
                    lhsT=pots[2][:, tb4_ * 128:(tb4_ + 1) * 128],
                    rhs=wp_sb[:, 2, half_ * CH:(half_ + 1) * CH],
                    start=False,
                    stop=True,
                )
                ost = osp.tile([128, CH], f32, tag=f"ost{half_}", name="ost")
                nc.vector.tensor_copy(ost, pp_[:, 0:CH])
                tb = pc * 4 + tb4_
                nc.sync.dma_start(
                    out=out_d[tb * 128:(tb + 1) * 128,
                              half_ * CH:(half_ + 1) * CH],
                    in_=ost,
                )

            prev_item = None
            for tb4 in range(4):
                for half in range(2):
                    pp = psP.tile([128, 512], f32, tag="pj", name="pspj")
                    for p in range(2):
                        nc.tensor.matmul(
                            pp[:, 0:CH],
                            lhsT=pots[p][:, tb4 * 128:(tb4 + 1) * 128],
                            rhs=wp_sb[:, p, half * CH:(half + 1) * CH],
                            start=(p == 0),
                            stop=False,
                        )
                    if prev_item is not None:
                        cfinish(prev_item)
                    prev_item = (pp, tb4, half)
            cfinish(prev_item)

    nc.compile()
    return nc


def _bf16(a):
    import ml_dtypes
    return np.ascontiguousarray(a).astype(ml_dtypes.bfloat16)


def _shard_inputs(x, W_attn, b_attn, W_proj):
    in_maps = []
    for c in range(8):
        b, hg = c // 2, c % 2
        q0, k0, v0 = hg * CH, C + hg * CH, 2 * C + hg * CH
        # per-head bqK column: (Wk_h @ bq_h) -> scores bias via exp-fold
        bcols = np.stack(
            [
                W_attn[:, k0 + h * D:k0 + (h + 1) * D]
                @ b_attn[q0 + h * D:q0 + (h + 1) * D]
                for h in range(NH)
            ],
            axis=1,
        )  # [C, 6]
        in_maps.append({
            "xt": _bf16(x[b].T),
            "wqk": _bf16(np.concatenate(
                [blkk
                 for p in range(NPAIR)
                 for blkk in (W_attn[:, q0 + p * 128:q0 + (p + 1) * 128],
                              W_attn[:, k0 + p * 128:k0 + (p + 1) * 128])],
                axis=1)),
            "wv": _bf16(np.concatenate(
                [W_attn[:, v0:v0 + CH], bcols], axis=1)),
            "wp": _bf16(
                W_proj[hg * CH:(hg + 1) * CH, :]
                .reshape(NPAIR, 128, C)
                .transpose(1, 0, 2)
                .reshape(128, NPAIR * C)
            ),
        })
    return in_maps


def kernel(x, W_attn, b_attn, W_proj, b_proj, _trace=False):
    from concourse.bass_utils import run_bass_kernel_spmd

    x = np.asarray(x, dtype=np.float32)
    W_attn = np.asarray(W_attn, dtype=np.float32)
    b_attn = np.asarray(b_attn, dtype=np.float32)
    W_proj = np.asarray(W_proj, dtype=np.float32)
    b_proj = np.asarray(b_proj, dtype=np.float32)

    if "nc" not in _CACHE:
        _CACHE["nc"] = _build_nc()
    nc = _CACHE["nc"]

    in_maps = _shard_inputs(x, W_attn, b_attn, W_proj)
    res = run_bass_kernel_spmd(nc, in_maps, list(range(8)), trace=_trace)
    _CACHE["last_result"] = res

    # V-bias contribution is a constant row: b_eff = b_proj + b_v @ W_proj
    b_eff = b_proj + b_attn[2 * C:] @ W_proj
    out = np.empty((B, T, C), dtype=np.float32)
    for b in range(B):
        out[b] = res.results[2 * b]["out"] + res.results[2 * b + 1]["out"] + b_eff
    return out


# revision 39
# speedup vs baseline: 1.0066x; 1.0066x over previous
"""Causal self-attention (B=4, T=2048, C=768, H=12) on 8 TRN2 NeuronCores.

Sharding: (batch x head-half). Core c handles batch b = c//2 and heads
hg*6..hg*6+5 where hg = c%2. Each core computes the qkv projection for its
1152 W_attn columns, causal attention for its 6 heads, and a partial
c_proj using its 384 rows of W_proj. Host sums the pair partials + b_eff.

Software-pipelined single-stream structure (~200us, vs 248us baseline):
- Query chunks processed in order [0, 3, 2, 1]: chunk 0 first (smallest
  DMA prerequisite set -> earliest start), the exp-heaviest chunk second,
  a light chunk last (ScalarE exp deficit never leaves the PE idle at the
  end).
- Q/K/V/c_proj are decoupled into small work items emitted just-in-time
  before the attention pti that needs them, or earlier as "fillers" paced
  by a debt ledger into the ACT-bound attention inner loop.
- Score matmuls of a head pair (h2=0 on PE rows 0-63, h2=1 on rows
  64-127) write one shared PSUM tile (h2=1 at column 512) so both release
  on the same exp and issue back-to-back -> they overlap in the PE array.
- PV matmuls are deferred TWO ptis so their pt operands (exp + causal
  mask) are always long done -> no pipeline-restart penalty.
- Pair normalization is deferred one pair and flushed after the next
  pair's jit projections: its vector-queue ops never delay the qkT casts
  (cross-engine deps are queue-prefix-ordered counting semaphores).
- Causal masking of diagonal blocks = VectorE multiply with a
  precomputed 0/1 triangle (gpsimd's queue would otherwise gate pt reuse
  behind slow partition_broadcasts).
- wqk is stored pair-major host-side and the input DMAs are staged
  (pair-0 weights + first xt chunk first) so the first matmul starts
  ~8us in instead of ~18us.
- Bias algebra: K-projection bias dropped (softmax-invariant), V bias
  folded host-side into b_eff = b_proj + b_v @ W_proj, Q bias enters as
  exp(bqK/8) folded into the PV stationary operand (col 64 of the
  stationary = scaled ones -> row 64 of the PV accumulator is the softmax
  denominator).
- c_proj is split into two 384-wide halves (1 PSUM bank each), deferred
  one chunk as filler work; the last chunk's c_proj is software-pipelined
  so pair-0/1 partials run under the final normalization wait.
"""

import sys

import numpy as np

try:
    import concourse  # noqa: F401
except ImportError:
    sys.path.insert(0, "/opt/trn_rl_repo")

B, T, C, H, D = 4, 2048, 768, 12, 64
NH = H // 2          # 6 heads per core
CH = NH * D          # 384 channels per core
NCB = C // 128       # 6 contraction blocks
NTB = T // 128       # 16 t-blocks
NQC = T // 512       # 4 query chunks
NPAIR = NH // 2      # 3 head pairs
VW2 = D + 2          # 66: [V(64), eb, pad] per head (col 64 = eb = exp(bqK/8))
VROW = NH * VW2      # 396
CORDER = [0, 3, 2, 1]

_CACHE = {}


def _build_nc():
    from concourse import bacc, mybir, tile

    f32 = mybir.dt.float32
    bf16 = mybir.dt.bfloat16
    AF = mybir.ActivationFunctionType
    ALU = mybir.AluOpType

    nc = bacc.Bacc("TRN2", target_bir_lowering=False, debug=False, num_devices=8)

    xt_d = nc.dram_tensor("xt", [C, T], bf16, kind="ExternalInput")
    wqk_d = nc.dram_tensor("wqk", [C, 2 * CH], bf16, kind="ExternalInput")
    wv_d = nc.dram_tensor("wv", [C, CH + NH], bf16, kind="ExternalInput")
    wp_d = nc.dram_tensor("wp", [128, NPAIR * C], bf16, kind="ExternalInput")
    out_d = nc.dram_tensor("out", [T, C], f32, kind="ExternalOutput")

    with tile.TileContext(nc) as tc:
        with (
            tc.tile_pool(name="const", bufs=1) as cp,
            tc.tile_pool(name="wk", bufs=3) as wk,
            tc.tile_pool(name="pt", bufs=4) as ptp,
            tc.tile_pool(name="ot", bufs=2) as otp,
            tc.tile_pool(name="outs", bufs=2) as osp,
            tc.tile_pool(name="ps", bufs=2, space="PSUM") as psS,
            tc.tile_pool(name="pj", bufs=2, space="PSUM") as psP,
            tc.tile_pool(name="pv", bufs=2, space="PSUM") as psV,
        ):
            # ---- resident inputs (full-width rows: max DMA run length) ----
            xt_r = xt_d.rearrange("(n p) m -> n p m", p=128)
            wqk_r = wqk_d.rearrange("(n p) m -> n p m", p=128)
            wv_r = wv_d.rearrange("(n p) m -> n p m", p=128)
            # wqk is PAIR-MAJOR host-side: [Q0|K0|Q1|K1|Q2|K2] x128 cols.
            # Stream order: pair-0 weights + first query chunk of xt first so
            # the projection pipeline starts early and is paced per ci block.
            xt_t, wqk_t, wv_t = [], [], []
            for ci in range(NCB):
                t_ = cp.tile([128, 2 * CH], bf16, tag=f"wqk{ci}", name=f"wqk{ci}")
                nc.sync.dma_start(out=t_[:, 0:256], in_=wqk_r[ci][:, 0:256])
                wqk_t.append(t_)
                t_ = cp.tile([128, T], bf16, tag=f"xt{ci}", name=f"xt{ci}")
                nc.sync.dma_start(out=t_[:, 0:512], in_=xt_r[ci][:, 0:512])
                xt_t.append(t_)
            for ci in range(NCB):
                t_ = cp.tile([128, CH + NH], bf16, tag=f"wv{ci}", name=f"wv{ci}")
                nc.sync.dma_start(out=t_, in_=wv_r[ci])
                wv_t.append(t_)
            for ci in range(NCB):
                nc.sync.dma_start(out=wqk_t[ci][:, 256:512],
                                  in_=wqk_r[ci][:, 256:512])
            for ci in range(NCB):
                nc.sync.dma_start(out=wqk_t[ci][:, 512:768],
                                  in_=wqk_r[ci][:, 512:768])
            for ci in range(NCB):
                nc.sync.dma_start(out=xt_t[ci][:, 512:T],
                                  in_=xt_r[ci][:, 512:T])
            wp_sb = cp.tile([128, NPAIR, C], bf16, tag="wp", name="wp")
            nc.sync.dma_start(out=wp_sb, in_=wp_d.rearrange("p (n m) -> p n m", n=NPAIR))

            def wqk_blk(blk, ci):
                return wqk_t[ci][:, blk * 128:(blk + 1) * 128]

            def xt_cols(ci, c0, c1):
                return xt_t[ci][:, c0:c1]

            # causal 0/1 mask for diagonal 128x128 blocks, built once so the
            # per-block masking runs on VectorE (gpsimd's prefix-ordered queue
            # otherwise gates pt reuse behind slow partition_broadcasts)
            tri = cp.tile([128, 128], bf16, tag="tri", name="tri")
            nc.gpsimd.memset(tri, 1.0)
            nc.gpsimd.affine_select(
                out=tri, in_=tri, compare_op=ALU.is_ge, fill=0.0, base=0,
                pattern=[[1, 128]], channel_multiplier=-1,
            )

            qkT = cp.tile([128, 6, T], bf16, tag="qkT", name="qkT")  # 0-2: Q, 3-5: K
            v1 = cp.tile([128, NTB, VROW], bf16, tag="v1", name="v1")
            v1_4d = v1.rearrange("p n (h e) -> p n h e", e=VW2)

            # ---------- work items ----------
            done_qk = {}       # (co, tc) -> True   co 0-2 Q-pair, 3-5 K-pair
            done_v = {}        # tb -> True

            def emit_proj(co, tcn):
                """Q or K projection for pair-column co, token chunk tcn."""
                if done_qk.get((co, tcn)):
                    return 0
                done_qk[(co, tcn)] = True
                blk = 2 * co if co < NPAIR else 2 * (co - NPAIR) + 1
                ps = psP.tile([128, 512], f32, tag="pj", name="pspj")
                for ci in range(NCB):
                    nc.tensor.matmul(
                        ps,
                        lhsT=wqk_blk(blk, ci),
                        rhs=xt_cols(ci, tcn * 512, (tcn + 1) * 512),
                        start=(ci == 0),
                        stop=(ci == NCB - 1),
                    )
                nc.vector.tensor_copy(qkT[:, co, tcn * 512:(tcn + 1) * 512], ps)
                return 1450

            def emit_v(tb):
                """V (+ bqK) projection for key t-block tb."""
                if done_v.get(tb):
                    return 0
                done_v[tb] = True
                psv = psP.tile([128, 512], f32, tag="pj", name="pspj")
                for ci in range(NCB):
                    nc.tensor.matmul(
                        psv[:, 0:CH + NH],
                        lhsT=xt_cols(ci, tb * 128, (tb + 1) * 128),
                        rhs=wv_t[ci],
                        start=(ci == 0),
                        stop=(ci == NCB - 1),
                    )
                eb = wk.tile([128, NH], f32, tag="eb", name="eb")
                nc.scalar.activation(eb, psv[:, CH:CH + NH], AF.Exp, scale=0.125)
                eb3 = eb.rearrange("p (h o) -> p h o", o=1)
                nc.vector.tensor_mul(
                    v1_4d[:, tb, :, 0:D],
                    psv[:, 0:CH].rearrange("p (h e) -> p h e", e=D),
                    eb3.to_broadcast([128, NH, D]),
                )
                nc.vector.tensor_copy(v1_4d[:, tb, :, D:D + 1], eb3)
                return 1150

            def emit_cproj_half(c, tb4, half, ot_tiles):
                """c_proj for t-block c*4+tb4, output columns half*384:+384."""
                tb = c * 4 + tb4
                pp = psP.tile([128, 512], f32, tag="pj", name="pspj")
                for p in range(NPAIR):
                    nc.tensor.matmul(
                        pp[:, 0:CH],
                        lhsT=ot_tiles[p][:, tb4 * 128:(tb4 + 1) * 128],
                        rhs=wp_sb[:, p, half * CH:(half + 1) * CH],
                        start=(p == 0),
                        stop=(p == NPAIR - 1),
                    )
                ost = osp.tile([128, CH], f32, tag=f"ost{half}", name="ost")
                nc.vector.tensor_copy(ost, pp[:, 0:CH])
                nc.sync.dma_start(
                    out=out_d[tb * 128:(tb + 1) * 128, half * CH:(half + 1) * CH],
                    in_=ost,
                )
                return 680

            cfill = []         # c_proj closures (must drain one chunk ahead)
            fillers = []       # proj/V closures returning pe-ns
            debt = [0.0]
            ot_map = {}        # chunk -> [otpair per pair]
            norm_pending = [None]

            def flush_norm():
                if norm_pending[0] is None:
                    return
                ps_pv, ots, p = norm_pending[0]
                norm_pending[0] = None
                otpair = otp.tile([128, 512], bf16, tag=f"ot{p}",
                                  name=f"ot{p}")
                for h2 in range(2):
                    # sums live at partition 64; custom-DVE ops misread
                    # base-64 APs, so standard-copy to partition 0 first
                    sums_sb = wk.tile([1, 512], f32, tag="sums", name="sums")
                    nc.vector.tensor_copy(sums_sb, ps_pv[h2][D:D + 1, :])
                    rb1 = wk.tile([1, 512], f32, tag="rb1", name="rb1")
                    nc.vector.reciprocal_approx_fast(rb1, sums_sb)
                    rbb = wk.tile([64, 512], f32, tag="rbb", name="rbb")
                    nc.gpsimd.partition_broadcast(rbb, rb1)
                    nc.vector.tensor_mul(
                        otpair[h2 * 64:(h2 + 1) * 64, :],
                        ps_pv[h2][0:D, :],
                        rbb,
                    )
                ots[p] = otpair

            def pump(ns):
                debt[0] += ns
                while debt[0] > 0 and (cfill or fillers):
                    q = cfill if cfill else fillers
                    debt[0] -= q.pop(0)()

            def emit_pv(nc_, pts, ps_pv, p, pti, j, nkb):
                for half, kb in ((0, 2 * pti), (1, 2 * pti + 1)):
                    pt, wd = pts[half]
                    qlo = 512 - wd
                    for h2 in range(2):
                        nc_.tensor.matmul(
                            ps_pv[h2][0:1 + D, qlo:512],
                            lhsT=v1_4d[:, kb, 2 * p + h2, 0:1 + D],
                            rhs=pt[:, h2 * 512:h2 * 512 + wd],
                            start=(kb == 0),
                            stop=(kb == nkb - 1),
                        )

            # ---------- main stream ----------
            pending = None     # (chunk, ot_tiles) whose c_proj is deferred
            for ic, j in enumerate(CORDER):
                nkb = 4 * (j + 1)
                # c_proj fillers of the chunk-before-last MUST be fully
                # emitted before this chunk's normalization recycles the ot
                # buffers (otp bufs=2), else PE-queue/WAR cycle -> deadlock
                for f in cfill:
                    f()
                cfill.clear()
                # queue fillers: deferred c_proj, then next chunk's prereqs
                if pending is not None:
                    pc, pots = pending
                    for tb4 in range(4):
                        for half in range(2):
                            cfill.append(
                                lambda c=pc, t=tb4, hf=half, o=pots:
                                emit_cproj_half(c, t, hf, o)
                            )
                    pending = None
                if ic + 1 < len(CORDER):
                    cn = CORDER[ic + 1]
                    for p in range(NPAIR):
                        for tcn in range(cn + 1):
                            if not done_qk.get((3 + p, tcn)):
                                fillers.append(
                                    lambda co=3 + p, t=tcn: emit_proj(co, t))
                        if not done_qk.get((p, cn)):
                            fillers.append(lambda co=p, t=cn: emit_proj(co, t))
                    for tb in range(4 * (cn + 1)):
                        if not done_v.get(tb):
                            fillers.append(lambda t=tb: emit_v(t))

                ot_map[j] = [None] * NPAIR
                for p in range(NPAIR):
                    # jit prereqs for this pair
                    debt[0] -= emit_proj(p, j)
                    for tcn in range(j + 1):
                        debt[0] -= emit_proj(3 + p, tcn)
                    flush_norm()
                    ps_pv = [
                        psV.tile([128, 512], f32, tag="pv", name=f"pspv{h2}")
                        for h2 in range(2)
                    ]
                    prevq = []   # pt tiles awaiting PV, deferred TWO ptis
                    pw = 1024.0
                    for pti in range(nkb // 2):
                        kb0, kb1 = 2 * pti, 2 * pti + 1
                        d0 = kb0 * 128 - j * 512
                        d1 = d0 + 128
                        qlo0, qlo1 = max(d0, 0), max(d1, 0)
                        w0, w1 = 512 - qlo0, 512 - qlo1
                        # one psum tile per key block holding BOTH h2 halves
                        # (h2=1 at column 512) so the paired score matmuls
                        # release together and issue back-to-back
                        pss = [
                            psS.tile([128, 1024], f32, tag="s", name=f"pss{kk}")
                            for kk in range(2)
                        ]
                        for kk, (kb, wd, ql) in enumerate(
                                ((kb0, w0, qlo0), (kb1, w1, qlo1))):
                            for h2 in range(2):
                                hp = h2 * 64
                                nc.tensor.matmul(
                                    pss[kk][:, h2 * 512:h2 * 512 + wd],
                                    lhsT=qkT[hp:hp + 64, 3 + p,
                                             kb * 128:(kb + 1) * 128],
                                    rhs=qkT[hp:hp + 64, p,
                                            j * 512 + ql:(j + 1) * 512],
                                    start=True,
                                    stop=True,
                                )
                        cur = []
                        for kk, (kb, wd, dd) in enumerate(
                                ((kb0, w0, d0), (kb1, w1, d1))):
                            pt = ptp.tile([128, 1024], bf16, tag=f"pt{kk}",
                                          name=f"pt{kk}")
                            if wd == 512:
                                nc.scalar.activation(
                                    pt, pss[kk], AF.Exp, scale=0.125)
                            else:
                                for h2 in range(2):
                                    nc.scalar.activation(
                                        pt[:, h2 * 512:h2 * 512 + wd],
                                        pss[kk][:, h2 * 512:h2 * 512 + wd],
                                        AF.Exp, scale=0.125,
                                    )
                            if dd >= 0:
                                for h2 in range(2):
                                    nc.vector.tensor_mul(
                                        pt[:, h2 * 512:h2 * 512 + 128],
                                        pt[:, h2 * 512:h2 * 512 + 128],
                                        tri,
                                    )
                            cur.append((pt, wd))
                        # V for these key blocks (consumed by a LATER pti's
                        # PV): emitted after the scores so the exp stream is
                        # never blocked behind V-proj (which gates on wv DMA)
                        debt[0] -= emit_v(kb0)
                        debt[0] -= emit_v(kb1)
                        # deficit: ACT exp time minus attention PE time this pti
                        sw = w0 + w1
                        prevq.append((cur, pti))
                        # drain to depth 1 at the pair's last pti: halves the
                        # pair-end PV backlog so the normalization chain (which
                        # gates the next pair's PV start via the psV buffer
                        # WAR) begins earlier
                        limit = 1 if pti == nkb // 2 - 1 else 2
                        popped = False
                        while len(prevq) > limit:
                            pts, pv_pti = prevq.pop(0)
                            emit_pv(nc, pts, ps_pv, p, pv_pti, j, nkb)
                            popped = True
                        if popped:
                            pump(1.25 * sw + 358 - 0.833 * pw)
                        else:
                            pump(1.25 * sw + 358)
                        pw = sw
                    for pts, pv_pti in prevq:
                        emit_pv(nc, pts, ps_pv, p, pv_pti, j, nkb)
                    # normalization is deferred one pair (flushed after the
                    # NEXT pair's jit projections) so its vector-queue ops
                    # never block the qkT casts the next scores need
                    norm_pending[0] = (ps_pv, ot_map[j], p)
                pending = (j, ot_map[j])

            # drain leftovers, then the last chunk's c_proj:
            # software-pipelined so each item's pair-0/1 matmuls run while
            # the previous item waits on the final pair-2 normalization
            flush_norm()
            for f in cfill:
                f()
            for f in fillers:
                f()
            pc, pots = pending

            def cfinish(item):
                pp_, tb4_, half_ = item
                nc.tensor.matmul(
                    pp_[:, 0:CH],
                    lhsT=pots[2][:, tb4_ * 128:(tb4_ + 1) * 128],
                    rhs=wp_sb[:, 2, half_ * CH:(half_ + 1) * CH],
                    start=False,
                    stop=True,
                )
                ost = osp.tile([128, CH], f32, tag=f"ost{half_}", name="ost")
                nc.vector.tensor_copy(ost, pp_[:, 0:CH])
                tb = pc * 4 + tb4_
                nc.sync.dma_start(
                    out=out_d[tb * 128:(tb + 1) * 128,
                              half_ * CH:(half_ + 1) * CH],
                    in_=ost,
                )

            # first two items borrow score-pool tiles: psS is idle after the
            # last exp, so their WAR is already satisfied, while psP's last
            # users' evacuation copies are still queued deep in VectorE
            prev_item = None
            for k, (tb4, half) in enumerate(
                    (t, h) for t in range(4) for h in range(2)):
                if k < 2:
                    pp = psS.tile([128, 1024], f32, tag="s", name="pscp")
                else:
                    pp = psP.tile([128, 512], f32, tag="pj", name="pspj")
                for p in range(2):
                    nc.tensor.matmul(
                        pp[:, 0:CH],
                        lhsT=pots[p][:, tb4 * 128:(tb4 + 1) * 128],
                        rhs=wp_sb[:, p, half * CH:(half + 1) * CH],
                        start=(p == 0),
                        stop=False,
                    )
                if prev_item is not None:
                    cfinish(prev_item)
                prev_item = (pp, tb4, half)
            cfinish(prev_item)

    nc.compile()
    return nc


def _bf16(a):
    import ml_dtypes
    return np.ascontiguousarray(a).astype(ml_dtypes.bfloat16)


def _shard_inputs(x, W_attn, b_attn, W_proj):
    in_maps = []
    for c in range(8):
        b, hg = c // 2, c % 2
        q0, k0, v0 = hg * CH, C + hg * CH, 2 * C + hg * CH
        # per-head bqK column: (Wk_h @ bq_h) -> scores bias via exp-fold
        bcols = np.stack(
            [
                W_attn[:, k0 + h * D:k0 + (h + 1) * D]
                @ b_attn[q0 + h * D:q0 + (h + 1) * D]
                for h in range(NH)
            ],
            axis=1,
        )  # [C, 6]
        in_maps.append({
            "xt": _bf16(x[b].T),
            "wqk": _bf16(np.concatenate(
                [blkk
                 for p in range(NPAIR)
                 for blkk in (W_attn[:, q0 + p * 128:q0 + (p + 1) * 128],
                              W_attn[:, k0 + p * 128:k0 + (p + 1) * 128])],
                axis=1)),
            "wv": _bf16(np.concatenate(
                [W_attn[:, v0:v0 + CH], bcols], axis=1)),
            "wp": _bf16(
                W_proj[hg * CH:(hg + 1) * CH, :]
                .reshape(NPAIR, 128, C)
                .transpose(1, 0, 2)
                .reshape(128, NPAIR * C)
            ),
        })
    return in_maps


def kernel(x, W_attn, b_attn, W_proj, b_proj, _trace=False):
    from concourse.bass_utils import run_bass_kernel_spmd

    x = np.asarray(x, dtype=np.float32)
    W_attn = np.asarray(W_attn, dtype=np.float32)
    b_attn = np.asarray(b_attn, dtype=np.float32)
    W_proj = np.asarray(W_proj, dtype=np.float32)
    b_proj = np.asarray(b_proj, dtype=np.float32)

    if "nc" not in _CACHE:
        _CACHE["nc"] = _build_nc()
    nc = _CACHE["nc"]

    in_maps = _shard_inputs(x, W_attn, b_attn, W_proj)
    res = run_bass_kernel_spmd(nc, in_maps, list(range(8)), trace=_trace)
    _CACHE["last_result"] = res

    # V-bias contribution is a constant row: b_eff = b_proj + b_v @ W_proj
    b_eff = b_proj + b_attn[2 * C:] @ W_proj
    out = np.empty((B, T, C), dtype=np.float32)
    for b in range(B):
        out[b] = res.results[2 * b]["out"] + res.results[2 * b + 1]["out"] + b_eff
    return out
